# revision 19
# baseline (speedup 1.0000x reference)
"""Trainium2 Bass kernel for CrossAttention3D (single-head, 512-dim, 4x64x64).

Algebraic restructure (per batch, X = q[b] viewed (C, S)):
    Softmax logits are invariant to per-query additive constants, so with
    qp = Wq X + bq, kp = Wk X + bk:
        logits[q,k] = scale * qp_q . kp_k
                    = (scale Wk^T Wq X_q) . X_k + beta_k  (+ per-q const, dropped)
        beta_k      = scale * (Wk^T bq) . X_k             (+ const, dropped)
    and the V/O projections commute through the (linear) attention average:
        out = (Wo Wv)(X E)/den + (Wo bv + bo),  E = exp(logits), den = 1^T E.
    Only TWO device projections remain: the fused QK projection applied on
    the query side and the fused VO projection after attention. beta rides
    along as the exp's per-partition bias.

All heavy matmuls are fp8 e4m3 with DoubleRow perf mode. Rescales keep
operands in e4m3's normal range: M0' = 32*scale*Wk^T Wq, N0' = 32*Wo Wv,
exp scale=1/32, XE evicted with scale 1/16, den "ones" = 2.0, so
32*(1/16)/2 = 1 net.

Keys are rotated per-core host-side so this core's queries are always
columns [0, QH) of x -- the QK projection reads its rhs straight out of
the key tiles and no separate query tensor is ever loaded.

Schedule: per query block, sim leads and den/AV consume et one iteration
behind. Initial loads ride three DMA queues (sync: m0+x8, gpsimd: xt,
scalar: beta/n0/b2/ones) so the first projection starts ~2us after the
framework preamble, and a burst of throwaway matmuls warms the PE HAM
clock gate during the DMA wait. The reciprocal broadcast runs in bf16
(single-pass matmul), VO results are normalized straight out of PSUM on
the DVE (no ACT eviction), and the qproj/VO PSUM tiles ride separate
pools so no block-boundary matmul waits on an ACT eviction.

Sharding: 8 cores = 4 batches x 2 query-halves, no collectives.
"""

import numpy as np
import ml_dtypes

import concourse.bass as bass
import concourse.bacc as bacc
import concourse.tile as tile
from concourse import mybir
from concourse.bass_utils import run_bass_kernel_spmd

AF = mybir.ActivationFunctionType
DR = mybir.MatmulPerfMode.DoubleRow
F32 = mybir.dt.float32
BF16 = mybir.dt.bfloat16
F8 = mybir.dt.float8e4

B, C, H, W = 4, 512, 64, 64
S = H * W            # 4096 tokens
P = 128              # partitions
CC = C // P          # 4 channel chunks
QH = S // 2          # 2048 queries per core
FB = 512             # query block
NQB = QH // FB       # 4 query blocks per core
NKC = S // P         # 32 key chunks
NI = NKC // 2        # 16 key-pair iterations
NXT = S // FB        # 8 x8 tiles of FB columns
N_CORES = 8
N_WARM = 20          # PE warm-up matmuls during the initial DMA wait


def _build_bass() -> bass.Bass:
    nc = bacc.Bacc("TRN2", target_bir_lowering=False)

    # all weight/bias tensors are host-packed so every DMA reads
    # partition-contiguous rows (>=512B descriptors, line-rate)
    x_d = nc.dram_tensor("x", [C, S], F8, kind="ExternalInput")     # keys, fp8
    xt_d = nc.dram_tensor("xt", [S, C], F8, kind="ExternalInput")   # X^T, fp8
    m0_d = nc.dram_tensor("m0", [P, CC * C], F8, kind="ExternalInput")
    n0_d = nc.dram_tensor("n0", [P, CC * C], F8, kind="ExternalInput")
    # all 2.0; padded to stride 16 to satisfy dual-fp8 Ldweights alignment
    ones_d = nc.dram_tensor("ones2", [P, 32], F8, kind="ExternalInput")
    bb_d = nc.dram_tensor("bb", [P, NKC + CC], F32, kind="ExternalInput")
    out_d = nc.dram_tensor("out", [C, QH], BF16, kind="ExternalOutput")

    x_r = x_d[:, :].rearrange("(cc p) s -> p cc s", p=P)
    xt_r = xt_d[:, :].rearrange("(kc p) c -> p kc c", p=P)
    out_r = out_d[:, :].rearrange("(oc p) s -> p oc s", p=P)

    with tile.TileContext(nc) as tc:
        with (
            tc.tile_pool(name="consts", bufs=1) as consts,
            tc.tile_pool(name="kkqs", bufs=2) as kkqs,
            tc.tile_pool(name="ets", bufs=6) as ets,
            tc.tile_pool(name="esums", bufs=2) as esums,
            tc.tile_pool(name="xes", bufs=2) as xes,
            tc.tile_pool(name="smalls", bufs=4) as smalls,
            tc.tile_pool(name="outs", bufs=8) as outs,
            tc.tile_pool(name="psmm", bufs=4, space="PSUM") as psmm,   # 4 banks
            tc.tile_pool(name="psav", bufs=4, space="PSUM") as psav,   # 4 banks
        ):
            # ---- initial loads: three DMA queues in parallel. sync carries
            # ---- the matmul-critical m0 + x8 stream, gpsimd (SWDGE) the
            # ---- transpose, scalar the small exp/VO-side constants. ----
            warm_sb = consts.tile([P, P], F8, tag="warm")
            nc.gpsimd.memset(warm_sb, 0.0)

            # one ordered queue: every tile is issued in deadline order, so
            # the critical m0/x8_0 never share SDMA bandwidth with the
            # later xt stream (cross-queue packet round-robin starves Q1)
            m0_sb = consts.tile([P, CC, CC, P], F8, tag="m0")
            m0_r = m0_d[:, :].rearrange("p (cp oc o) -> p cp oc o", cp=CC, oc=CC)
            # two halves so the first DR weight pair is ready ~0.5us sooner
            nc.sync.dma_start(out=m0_sb[:, 0:2, :, :], in_=m0_r[:, 0:2, :, :])
            nc.sync.dma_start(out=m0_sb[:, 2:CC, :, :], in_=m0_r[:, 2:CC, :, :])
            x8_t = [consts.tile([P, CC, FB], F8, tag=f"x8_{i}", name=f"x8_{i}")
                    for i in range(NXT)]
            xt8_t = [consts.tile([P, NKC // 4, C], F8, tag=f"xt8_{i}", name=f"xt8_{i}")
                     for i in range(4)]

            def dma_x8(i):
                nc.sync.dma_start(out=x8_t[i], in_=x_r[:, :, i * FB:(i + 1) * FB])

            def dma_xt(c4):
                klo = c4 * (NKC // 4)
                nc.sync.dma_start(out=xt8_t[c4], in_=xt_r[:, klo:klo + NKC // 4, :])

            dma_x8(0)
            dma_x8(1)
            dma_xt(0)
            dma_x8(2)
            dma_x8(3)
            dma_xt(1)
            dma_x8(4)
            dma_x8(5)
            dma_xt(2)
            dma_x8(6)
            dma_x8(7)
            dma_xt(3)
            ones8_sb = consts.tile([P, 2, 16], F8, tag="ones8")
            nc.sync.dma_start(out=ones8_sb, in_=ones_d[:, :])
            n0_sb = consts.tile([P, CC, C], F8, tag="n0")
            nc.sync.dma_start(
                out=n0_sb, in_=n0_d[:, :].rearrange("p (cc o) -> p cc o", cc=CC))

            bb_sb = consts.tile([P, NKC + CC], F32, tag="bb")
            nc.scalar.dma_start(out=bb_sb, in_=bb_d[:, :])

            ones_sb = consts.tile([P, 1], BF16, tag="ones")
            nc.vector.memset(ones_sb, 2.0)
            ones1_sb = consts.tile([1, P], BF16, tag="ones1")
            nc.vector.memset(ones1_sb, 1.0)

            # ---- PE warm-up: throwaway matmuls so the HAM clock gate is at
            # ---- full rate by the time the first real operand lands ----
            warm_ps = psmm.tile([P, P], F32, tag="mm", name="warm")
            for _ in range(N_WARM):
                nc.tensor.matmul(warm_ps, lhsT=warm_sb[:], rhs=warm_sb[:],
                                 start=True, stop=True)

            def x8_key(cc, k0):
                # sim lhsT [128, 2(cc pair), P] for keys [k0, k0+P)
                t = x8_t[k0 // FB]
                off = k0 % FB
                return t[:, cc:cc + 2, off:off + P]

            def xt8_key(kc, c4):
                # AV lhsT [128, 2(kc pair), P] for channel chunk c4
                t = xt8_t[kc // 8]
                return t[:, kc % 8:kc % 8 + 2, c4 * P:(c4 + 1) * P]

            kkq_tiles = {}

            def emit_qproj(qb):
                # fused QK projection of query block qb (PE + ACT evict);
                # rhs comes straight from the key tiles (queries = cols 0..QH)
                kkq = kkqs.tile([P, CC, FB], F8, tag="kkq", name=f"kkq{qb}")
                for oc in range(CC):
                    ps = psmm.tile([P, FB], F32, tag="mm", name=f"qp{qb}_{oc}")
                    for ci, cc in enumerate(range(0, CC, 2)):
                        nc.tensor.matmul(
                            ps, lhsT=m0_sb[:, cc:cc + 2, oc, :],
                            rhs=x8_t[qb][:, cc:cc + 2, :],
                            start=(ci == 0), stop=(ci == 1), perf_mode=DR,
                        )
                    nc.scalar.activation(out=kkq[:, oc, :], in_=ps[:], func=AF.Copy)
                kkq_tiles[qb] = kkq

            emit_qproj(0)

            for qb in range(NQB):
                qsl = slice(qb * FB, (qb + 1) * FB)
                kkq = kkq_tiles[qb]
                last = qb == NQB - 1

                # ---- attention: sim leads; esum/AV consume et one iter behind ----
                avt = [psav.tile([P, FB], F32, tag="av", name=f"avt{qb}_{i}")
                       for i in range(CC)]
                esum = esums.tile([P, 2, FB], F32, tag="esum")
                ets_q = []

                def emit_sim(i, qb=qb, kkq=kkq):
                    simt = [psmm.tile([P, FB], F32, tag="mm", name=f"sim{qb}_{i}_{j}")
                            for j in range(2)]
                    for j in range(2):
                        k0 = (2 * i + j) * P
                        for ci, cc in enumerate(range(0, CC, 2)):
                            nc.tensor.matmul(
                                simt[j], lhsT=x8_key(cc, k0),
                                rhs=kkq[:, cc:cc + 2, :],
                                start=(ci == 0), stop=(ci == 1), perf_mode=DR,
                            )
                    et = ets.tile([P, 2, FB], F8, tag="et", name=f"et{qb}_{i}")
                    for j in range(2):
                        kc = 2 * i + j
                        nc.scalar.activation(
                            out=et[:, j, :], in_=simt[j][:], func=AF.Exp,
                            bias=bb_sb[:, kc:kc + 1], scale=1.0 / 32.0,
                        )
                    ets_q.append(et)

                den = [None]

                def emit_denav(i, qb=qb, avt=avt, esum=esum, ets_q=ets_q, den=den):
                    et = ets_q[i]
                    # denominator partials: iters 0..13 accumulate on the DVE
                    # (off the critical path); the last two go straight into
                    # the den PSUM with DR matmuls so den closes ~1us after
                    # the final exp. den is allocated lazily so the psmm ring
                    # never hands a sim tile a bank that den still holds.
                    if i == 0:
                        nc.vector.tensor_scalar_add(out=esum, in0=et[:, :, :], scalar1=0.0)
                    elif i <= NI - 3:
                        nc.vector.tensor_add(out=esum, in0=esum[:, :, :], in1=et[:, :, :])
                    else:
                        if i == NI - 2:
                            den[0] = psmm.tile([1, FB], F32, tag="mm", name=f"den{qb}")
                        nc.tensor.matmul(
                            den[0], lhsT=ones8_sb[:, :, 0:1], rhs=et[:, :, :],
                            start=(i == NI - 2), stop=False, perf_mode=DR,
                        )
                    for c4 in range(CC):
                        nc.tensor.matmul(
                            avt[c4], lhsT=xt8_key(2 * i, c4),
                            rhs=et[:, :, :],
                            start=(i == 0), stop=(i == NI - 1), perf_mode=DR,
                        )

                for i in range(NI):
                    emit_sim(i)
                    if i > 0:
                        emit_denav(i - 1)
                    if i == NI - 1:
                        # esbA (iters 0..13) evicts early on the DVE, keeping
                        # the ACT exp queue clear around the block boundary
                        esb = smalls.tile([P, 2, FB], BF16, tag="esb")
                        nc.vector.tensor_scalar_add(out=esb, in0=esum[:, :, :], scalar1=0.0)
                # last iteration: AV matmuls + den DR matmul, then XE evicts
                et15 = ets_q[NI - 1]
                for c4 in range(CC):
                    nc.tensor.matmul(
                        avt[c4], lhsT=xt8_key(2 * (NI - 1), c4),
                        rhs=et15[:, :, :],
                        start=False, stop=True, perf_mode=DR,
                    )
                nc.tensor.matmul(
                    den[0], lhsT=ones8_sb[:, :, 0:1], rhs=et15[:, :, :],
                    start=False, stop=False, perf_mode=DR,
                )
                nc.tensor.matmul(den[0], lhsT=ones_sb[:], rhs=esb[:, 0, :], start=False, stop=False)
                nc.tensor.matmul(den[0], lhsT=ones_sb[:], rhs=esb[:, 1, :], start=False, stop=True)

                # ---- reciprocal chain leads the DVE queue (it only waits on
                # ---- den's stop), then the XE evictions (gpsimd cannot read
                # ---- PSUM, so they must ride the DVE) ----
                rec = smalls.tile([1, FB], F32, tag="rec", name=f"rec{qb}")
                nc.vector.reciprocal_approx_fast(out=rec, in_=den[0][:])
                rec_bf = smalls.tile([1, FB], BF16, tag="recbf", name=f"recbf{qb}")
                nc.vector.tensor_scalar_add(out=rec_bf, in0=rec[:], scalar1=0.0)
                xe8 = xes.tile([P, CC, FB], F8, tag="xe8")
                for c4 in range(CC):
                    if last:
                        # exps are done by now: the idle ACT evicts XE so the
                        # DVE tail chain is just rec/rbc/normalize
                        nc.scalar.activation(out=xe8[:, c4, :], in_=avt[c4][:],
                                             func=AF.Copy, scale=1.0 / 16.0)
                    else:
                        nc.vector.tensor_scalar_mul(
                            out=xe8[:, c4, :], in0=avt[c4][:], scalar1=1.0 / 16.0)

                # next block's projection fills the PE while the reciprocal
                # broadcast is in flight
                if not last:
                    emit_qproj(qb + 1)

                # bf16 single-pass broadcast of 1/den to all partitions
                rbc = psmm.tile([P, FB], F32, tag="mm", name=f"rbc{qb}")
                nc.tensor.matmul(rbc, lhsT=ones1_sb[:], rhs=rec_bf[:],
                                 start=True, stop=True)
                rbc_sb = smalls.tile([P, FB], F32, tag="rbcsb", name=f"rbcsb{qb}")
                nc.vector.tensor_scalar_add(out=rbc_sb, in0=rbc[:], scalar1=0.0)

                # ---- fused VO projection; normalized straight out of PSUM on
                # ---- the DVE -- no ACT eviction anywhere on this path ----
                for oc in range(CC):
                    po = psav.tile([P, FB], F32, tag="av", name=f"po{qb}_{oc}")
                    for ci, cc in enumerate(range(0, CC, 2)):
                        nc.tensor.matmul(
                            po, lhsT=n0_sb[:, cc:cc + 2, oc * P:(oc + 1) * P],
                            rhs=xe8[:, cc:cc + 2, :],
                            start=(ci == 0), stop=(ci == 1), perf_mode=DR,
                        )
                    ot = outs.tile([P, FB], BF16, tag="ot")
                    nc.vector.tensor_mul(out=ot, in0=po[:], in1=rbc_sb[:])
                    nc.vector.tensor_scalar_add(
                        out=ot, in0=ot[:], scalar1=bb_sb[:, NKC + oc:NKC + oc + 1])
                    # last block: alternate out-DMA queues (ACT is idle then)
                    eng = nc.scalar if (last and oc % 2 == 1) else nc.sync
                    eng.dma_start(out=out_r[:, oc, qsl], in_=ot[:])

    nc.finalize()
    return nc


_NC_CACHE = {}


def _get_nc() -> bass.Bass:
    if "nc" not in _NC_CACHE:
        _NC_CACHE["nc"] = _build_bass()
    return _NC_CACHE["nc"]


def make_in_maps(q, Wq, bq, Wk, bk, Wv, bv, Wo, bo):
    f = np.float32
    f8 = ml_dtypes.float8_e4m3
    scale = f(C) ** f(-0.5)

    def q8(a):
        return np.ascontiguousarray(
            np.clip(np.asarray(a, f), -240, 240).astype(f8))

    Wq, Wk, Wv, Wo = (np.asarray(a, f) for a in (Wq, Wk, Wv, Wo))
    bq, bk, bv, bo = (np.asarray(a, f) for a in (bq, bk, bv, bo))
    # pack weights partition-major so each DMA row is one contiguous run:
    # m0[p, cp, oc, o] = M0'.T[cp*P+p, oc*P+o], flattened to [P, CC*C]
    m0 = np.ascontiguousarray(
        q8(((Wk.T @ Wq) * (32.0 * scale)).T)
        .reshape(CC, P, CC, P).transpose(1, 0, 2, 3).reshape(P, CC * C))
    # n0[p, cc, o] = (32*Wo@Wv).T[cc*P+p, o], flattened to [P, CC*C]
    n0 = np.ascontiguousarray(
        q8((32.0 * (Wo @ Wv)).T)
        .reshape(CC, P, C).transpose(1, 0, 2).reshape(P, CC * C))
    ones2 = np.full((P, 32), 2.0, f8)
    w_beta = (Wk.T @ bq) * scale
    b2 = (Wo @ bv + bo).astype(f)

    in_maps = []
    for core in range(N_CORES):
        b, half = core // 2, core % 2
        X = np.asarray(q[b], f).reshape(C, S)
        # rotate keys so this core's queries are always columns [0, QH);
        # softmax is invariant to a consistent key permutation
        Xr = np.concatenate([X[:, half * QH:], X[:, :half * QH]], axis=1)
        x8 = q8(Xr)
        beta = (w_beta @ Xr).astype(f)
        # bb[p, kc] = beta[kc*P+p]; bb[p, NKC+oc] = b2[oc*P+p]
        bb = np.concatenate(
            [beta.reshape(NKC, P).T, b2.reshape(CC, P).T], axis=1)
        in_maps.append({
            "x": x8,
            "xt": np.ascontiguousarray(x8.T),
            "m0": m0, "n0": n0, "ones2": ones2,
            "bb": np.ascontiguousarray(bb),
        })
    return in_maps


def gather_out(per_core_outs):
    out = np.zeros((B, C, S), np.float32)
    for core in range(N_CORES):
        b, half = core // 2, core % 2
        out[b, :, half * QH:(half + 1) * QH] = np.asarray(
            per_core_outs[core]).astype(np.float32)
    return out.reshape(B, C, H, W)


def kernel(q, Wq, bq, Wk, bk, Wv, bv, Wo, bo):
    nc = _get_nc()
    in_maps = make_in_maps(q, Wq, bq, Wk, bk, Wv, bv, Wo, bo)
    res = run_bass_kernel_spmd(nc, in_maps, core_ids=list(range(N_CORES)))
    return gather_out([res.results[i]["out"] for i in range(N_CORES)])


# revision 24
# speedup vs baseline: 1.0126x; 1.0126x over previous
"""Trainium2 Bass kernel for CrossAttention3D (single-head, 512-dim, 4x64x64).

Algebraic restructure (per batch, X = q[b] viewed (C, S)):
    Softmax logits are invariant to per-query additive constants, so with
    qp = Wq X + bq, kp = Wk X + bk:
        logits[q,k] = scale * qp_q . kp_k
                    = (scale Wk^T Wq X_q) . X_k + beta_k  (+ per-q const, dropped)
        beta_k      = scale * (Wk^T bq) . X_k             (+ const, dropped)
    and the V/O projections commute through the (linear) attention average:
        out = (Wo Wv)(X E)/den + (Wo bv + bo),  E = exp(logits), den = 1^T E.
    Only TWO device projections remain: the fused QK projection applied on
    the query side and the fused VO projection after attention. beta rides
    along as the exp's per-partition bias.

All heavy matmuls are fp8 e4m3 with DoubleRow perf mode. Rescales keep
operands in e4m3's normal range: M0' = 32*scale*Wk^T Wq, N0' = 32*Wo Wv,
exp scale=1/32, XE evicted with scale 1/16, den "ones" = 2.0, so
32*(1/16)/2 = 1 net.

Keys are rotated per-core host-side so this core's queries are always
columns [0, QH) of x -- the QK projection reads its rhs straight out of
the key tiles and no separate query tensor is ever loaded.

Schedule: per query block, sim leads and den/AV consume et one iteration
behind. Initial loads ride three DMA queues (sync: m0+x8, gpsimd: xt,
scalar: beta/n0/b2/ones) so the first projection starts ~2us after the
framework preamble, and a burst of throwaway matmuls warms the PE HAM
clock gate during the DMA wait. The reciprocal broadcast runs in bf16
(single-pass matmul), VO results are normalized straight out of PSUM on
the DVE (no ACT eviction), and the qproj/VO PSUM tiles ride separate
pools so no block-boundary matmul waits on an ACT eviction.

Sharding: 8 cores = 4 batches x 2 query-halves, no collectives.
"""

import numpy as np
import ml_dtypes

import concourse.bass as bass
import concourse.bacc as bacc
import concourse.tile as tile
from concourse import mybir
from concourse.bass_utils import run_bass_kernel_spmd

AF = mybir.ActivationFunctionType
DR = mybir.MatmulPerfMode.DoubleRow
F32 = mybir.dt.float32
BF16 = mybir.dt.bfloat16
F8 = mybir.dt.float8e4

B, C, H, W = 4, 512, 64, 64
S = H * W            # 4096 tokens
P = 128              # partitions
CC = C // P          # 4 channel chunks
QH = S // 2          # 2048 queries per core
FB = 512             # query block
NQB = QH // FB       # 4 query blocks per core
NKC = S // P         # 32 key chunks
NI = NKC // 2        # 16 key-pair iterations
NXT = S // FB        # 8 x8 tiles of FB columns
N_CORES = 8
N_WARM = 24          # PE warm-up matmuls during the initial DMA wait


def _build_bass() -> bass.Bass:
    nc = bacc.Bacc("TRN2", target_bir_lowering=False)

    # all weight/bias tensors are host-packed so every DMA reads
    # partition-contiguous rows (>=512B descriptors, line-rate)
    x_d = nc.dram_tensor("x", [C, S], F8, kind="ExternalInput")     # keys, fp8
    xt_d = nc.dram_tensor("xt", [S, C], F8, kind="ExternalInput")   # X^T, fp8
    m0_d = nc.dram_tensor("m0", [P, CC * C], F8, kind="ExternalInput")
    n0_d = nc.dram_tensor("n0", [P, CC * C], F8, kind="ExternalInput")
    # all 2.0; padded to stride 16 to satisfy dual-fp8 Ldweights alignment
    ones_d = nc.dram_tensor("ones2", [P, 32], F8, kind="ExternalInput")
    bb_d = nc.dram_tensor("bb", [P, NKC + CC], F32, kind="ExternalInput")
    out_d = nc.dram_tensor("out", [C, QH], BF16, kind="ExternalOutput")

    x_r = x_d[:, :].rearrange("(cc p) s -> p cc s", p=P)
    xt_r = xt_d[:, :].rearrange("(kc p) c -> p kc c", p=P)
    out_r = out_d[:, :].rearrange("(oc p) s -> p oc s", p=P)

    with tile.TileContext(nc) as tc:
        with (
            tc.tile_pool(name="consts", bufs=1) as consts,
            tc.tile_pool(name="kkqs", bufs=2) as kkqs,
            tc.tile_pool(name="ets", bufs=6) as ets,
            tc.tile_pool(name="esums", bufs=2) as esums,
            tc.tile_pool(name="xes", bufs=2) as xes,
            tc.tile_pool(name="smalls", bufs=4) as smalls,
            tc.tile_pool(name="outs", bufs=8) as outs,
            tc.tile_pool(name="psmm", bufs=4, space="PSUM") as psmm,   # 4 banks
            tc.tile_pool(name="psav", bufs=4, space="PSUM") as psav,   # 4 banks
        ):
            # ---- initial loads: three DMA queues in parallel. sync carries
            # ---- the matmul-critical m0 + x8 stream, gpsimd (SWDGE) the
            # ---- transpose, scalar the small exp/VO-side constants. ----
            warm_sb = consts.tile([P, P], F8, tag="warm")
            nc.gpsimd.memset(warm_sb, 0.0)

            # one ordered queue: every tile is issued in deadline order, so
            # the critical m0/x8_0 never share SDMA bandwidth with the
            # later xt stream (cross-queue packet round-robin starves Q1)
            m0_sb = consts.tile([P, CC, CC, P], F8, tag="m0")
            m0_r = m0_d[:, :].rearrange("p (cp oc o) -> p cp oc o", cp=CC, oc=CC)
            x8_t = [consts.tile([P, CC, FB], F8, tag=f"x8_{i}", name=f"x8_{i}")
                    for i in range(NXT)]
            xt8_t = [consts.tile([P, NKC // 4, C], F8, tag=f"xt8_{i}", name=f"xt8_{i}")
                     for i in range(4)]

            def dma_x8(i):
                nc.sync.dma_start(out=x8_t[i], in_=x_r[:, :, i * FB:(i + 1) * FB])

            def dma_xt(c4):
                klo = c4 * (NKC // 4)
                nc.sync.dma_start(out=xt8_t[c4], in_=xt_r[:, klo:klo + NKC // 4, :])

            # deadline order; the first qproj matmul gates on m0-half-1 + x8_0
            nc.sync.dma_start(out=m0_sb[:, 0:2, :, :], in_=m0_r[:, 0:2, :, :])
            dma_x8(0)
            nc.sync.dma_start(out=m0_sb[:, 2:CC, :, :], in_=m0_r[:, 2:CC, :, :])
            dma_x8(1)
            dma_xt(0)
            dma_x8(2)
            dma_x8(3)
            dma_xt(1)
            dma_x8(4)
            dma_x8(5)
            dma_xt(2)
            dma_x8(6)
            dma_x8(7)
            dma_xt(3)
            ones8_sb = consts.tile([P, 2, 16], F8, tag="ones8")
            nc.sync.dma_start(out=ones8_sb, in_=ones_d[:, :])
            n0_sb = consts.tile([P, CC, C], F8, tag="n0")
            nc.sync.dma_start(
                out=n0_sb, in_=n0_d[:, :].rearrange("p (cc o) -> p cc o", cc=CC))

            bb_sb = consts.tile([P, NKC + CC], F32, tag="bb")
            nc.scalar.dma_start(out=bb_sb, in_=bb_d[:, :])

            ones_sb = consts.tile([P, 1], BF16, tag="ones")
            nc.vector.memset(ones_sb, 2.0)
            ones1_sb = consts.tile([1, P], BF16, tag="ones1")
            nc.vector.memset(ones1_sb, 1.0)

            # ---- PE warm-up: throwaway matmuls so the HAM clock gate is at
            # ---- full rate by the time the first real operand lands ----
            warm_ps = psmm.tile([P, P], F32, tag="mm", name="warm")
            for _ in range(N_WARM):
                nc.tensor.matmul(warm_ps, lhsT=warm_sb[:], rhs=warm_sb[:],
                                 start=True, stop=True)

            def x8_key(cc, k0):
                # sim lhsT [128, 2(cc pair), P] for keys [k0, k0+P)
                t = x8_t[k0 // FB]
                off = k0 % FB
                return t[:, cc:cc + 2, off:off + P]

            def xt8_key(kc, c4):
                # AV lhsT [128, 2(kc pair), P] for channel chunk c4
                t = xt8_t[kc // 8]
                return t[:, kc % 8:kc % 8 + 2, c4 * P:(c4 + 1) * P]

            kkq_tiles = {}

            def emit_qproj(qb):
                # fused QK projection of query block qb (PE + ACT evict);
                # rhs comes straight from the key tiles (queries = cols 0..QH)
                kkq = kkqs.tile([P, CC, FB], F8, tag="kkq", name=f"kkq{qb}")
                for oc in range(CC):
                    ps = psmm.tile([P, FB], F32, tag="mm", name=f"qp{qb}_{oc}")
                    for ci, cc in enumerate(range(0, CC, 2)):
                        nc.tensor.matmul(
                            ps, lhsT=m0_sb[:, cc:cc + 2, oc, :],
                            rhs=x8_t[qb][:, cc:cc + 2, :],
                            start=(ci == 0), stop=(ci == 1), perf_mode=DR,
                        )
                    nc.scalar.activation(out=kkq[:, oc, :], in_=ps[:], func=AF.Copy)
                kkq_tiles[qb] = kkq

            emit_qproj(0)

            for qb in range(NQB):
                qsl = slice(qb * FB, (qb + 1) * FB)
                kkq = kkq_tiles[qb]
                last = qb == NQB - 1

                # ---- attention: sim leads; esum/AV consume et one iter behind ----
                avt = [psav.tile([P, FB], F32, tag="av", name=f"avt{qb}_{i}")
                       for i in range(CC)]
                esum = esums.tile([P, 2, FB], F32, tag="esum")
                ets_q = []

                def emit_sim(i, qb=qb, kkq=kkq):
                    simt = [psmm.tile([P, FB], F32, tag="mm", name=f"sim{qb}_{i}_{j}")
                            for j in range(2)]
                    for j in range(2):
                        k0 = (2 * i + j) * P
                        for ci, cc in enumerate(range(0, CC, 2)):
                            nc.tensor.matmul(
                                simt[j], lhsT=x8_key(cc, k0),
                                rhs=kkq[:, cc:cc + 2, :],
                                start=(ci == 0), stop=(ci == 1), perf_mode=DR,
                            )
                    et = ets.tile([P, 2, FB], F8, tag="et", name=f"et{qb}_{i}")
                    for j in range(2):
                        kc = 2 * i + j
                        nc.scalar.activation(
                            out=et[:, j, :], in_=simt[j][:], func=AF.Exp,
                            bias=bb_sb[:, kc:kc + 1], scale=1.0 / 32.0,
                        )
                    ets_q.append(et)

                den = [None]

                def emit_denav(i, qb=qb, avt=avt, esum=esum, ets_q=ets_q, den=den):
                    et = ets_q[i]
                    # denominator partials: iters 0..13 accumulate on the DVE
                    # (off the critical path); the last two go straight into
                    # the den PSUM with DR matmuls so den closes ~1us after
                    # the final exp. den is allocated lazily so the psmm ring
                    # never hands a sim tile a bank that den still holds.
                    if i == 0:
                        nc.vector.tensor_scalar_add(out=esum, in0=et[:, :, :], scalar1=0.0)
                    elif i <= NI - 3:
                        nc.vector.tensor_add(out=esum, in0=esum[:, :, :], in1=et[:, :, :])
                    else:
                        if i == NI - 2:
                            den[0] = psmm.tile([1, FB], F32, tag="mm", name=f"den{qb}")
                        nc.tensor.matmul(
                            den[0], lhsT=ones8_sb[:, :, 0:1], rhs=et[:, :, :],
                            start=(i == NI - 2), stop=False, perf_mode=DR,
                        )
                    for c4 in range(CC):
                        nc.tensor.matmul(
                            avt[c4], lhsT=xt8_key(2 * i, c4),
                            rhs=et[:, :, :],
                            start=(i == 0), stop=(i == NI - 1), perf_mode=DR,
                        )

                for i in range(NI):
                    emit_sim(i)
                    if i > 0:
                        emit_denav(i - 1)
                    if i == NI - 1:
                        # esbA (iters 0..13) evicts early, off the tail path
                        esb = smalls.tile([P, 2, FB], BF16, tag="esb")
                        nc.scalar.activation(out=esb, in_=esum[:, :, :], func=AF.Copy)
                # last iteration: AV matmuls + den DR matmul, then XE evicts
                et15 = ets_q[NI - 1]
                for c4 in range(CC):
                    nc.tensor.matmul(
                        avt[c4], lhsT=xt8_key(2 * (NI - 1), c4),
                        rhs=et15[:, :, :],
                        start=False, stop=True, perf_mode=DR,
                    )
                nc.tensor.matmul(
                    den[0], lhsT=ones8_sb[:, :, 0:1], rhs=et15[:, :, :],
                    start=False, stop=False, perf_mode=DR,
                )
                nc.tensor.matmul(den[0], lhsT=ones_sb[:], rhs=esb[:, 0, :], start=False, stop=False)
                nc.tensor.matmul(den[0], lhsT=ones_sb[:], rhs=esb[:, 1, :], start=False, stop=True)

                # ---- reciprocal chain leads the DVE queue (it only waits on
                # ---- den's stop), then the XE evictions (gpsimd cannot read
                # ---- PSUM, so they must ride the DVE) ----
                rec = smalls.tile([1, FB], F32, tag="rec", name=f"rec{qb}")
                nc.vector.reciprocal_approx_fast(out=rec, in_=den[0][:])
                rec_bf = smalls.tile([1, FB], BF16, tag="recbf", name=f"recbf{qb}")
                nc.vector.tensor_scalar_add(out=rec_bf, in0=rec[:], scalar1=0.0)
                xe8 = xes.tile([P, CC, FB], F8, tag="xe8")
                for c4 in range(CC):
                    nc.vector.tensor_scalar_mul(
                        out=xe8[:, c4, :], in0=avt[c4][:], scalar1=1.0 / 16.0)

                # next block's projection fills the PE while the reciprocal
                # broadcast is in flight
                if not last:
                    emit_qproj(qb + 1)

                # bf16 single-pass broadcast of 1/den to all partitions
                rbc = psmm.tile([P, FB], F32, tag="mm", name=f"rbc{qb}")
                nc.tensor.matmul(rbc, lhsT=ones1_sb[:], rhs=rec_bf[:],
                                 start=True, stop=True)
                rbc_sb = smalls.tile([P, FB], F32, tag="rbcsb", name=f"rbcsb{qb}")
                nc.vector.tensor_scalar_add(out=rbc_sb, in0=rbc[:], scalar1=0.0)

                # ---- fused VO projection; normalized straight out of PSUM on
                # ---- the DVE -- no ACT eviction anywhere on this path ----
                for oc in range(CC):
                    po = psav.tile([P, FB], F32, tag="av", name=f"po{qb}_{oc}")
                    for ci, cc in enumerate(range(0, CC, 2)):
                        nc.tensor.matmul(
                            po, lhsT=n0_sb[:, cc:cc + 2, oc * P:(oc + 1) * P],
                            rhs=xe8[:, cc:cc + 2, :],
                            start=(ci == 0), stop=(ci == 1), perf_mode=DR,
                        )
                    ot = outs.tile([P, FB], BF16, tag="ot")
                    nc.vector.tensor_mul(out=ot, in0=po[:], in1=rbc_sb[:])
                    nc.vector.tensor_scalar_add(
                        out=ot, in0=ot[:], scalar1=bb_sb[:, NKC + oc:NKC + oc + 1])
                    # last block: alternate out-DMA queues (ACT is idle then)
                    eng = nc.scalar if (last and oc % 2 == 1) else nc.sync
                    eng.dma_start(out=out_r[:, oc, qsl], in_=ot[:])

    nc.finalize()
    return nc


_NC_CACHE = {}


def _get_nc() -> bass.Bass:
    if "nc" not in _NC_CACHE:
        _NC_CACHE["nc"] = _build_bass()
    return _NC_CACHE["nc"]


def make_in_maps(q, Wq, bq, Wk, bk, Wv, bv, Wo, bo):
    f = np.float32
    f8 = ml_dtypes.float8_e4m3
    scale = f(C) ** f(-0.5)

    def q8(a):
        return np.ascontiguousarray(
            np.clip(np.asarray(a, f), -240, 240).astype(f8))

    Wq, Wk, Wv, Wo = (np.asarray(a, f) for a in (Wq, Wk, Wv, Wo))
    bq, bk, bv, bo = (np.asarray(a, f) for a in (bq, bk, bv, bo))
    # pack weights partition-major so each DMA row is one contiguous run:
    # m0[p, cp, oc, o] = M0'.T[cp*P+p, oc*P+o], flattened to [P, CC*C]
    m0 = np.ascontiguousarray(
        q8(((Wk.T @ Wq) * (32.0 * scale)).T)
        .reshape(CC, P, CC, P).transpose(1, 0, 2, 3).reshape(P, CC * C))
    # n0[p, cc, o] = (32*Wo@Wv).T[cc*P+p, o], flattened to [P, CC*C]
    n0 = np.ascontiguousarray(
        q8((32.0 * (Wo @ Wv)).T)
        .reshape(CC, P, C).transpose(1, 0, 2).reshape(P, CC * C))
    ones2 = np.full((P, 32), 2.0, f8)
    w_beta = (Wk.T @ bq) * scale
    b2 = (Wo @ bv + bo).astype(f)

    in_maps = []
    for core in range(N_CORES):
        b, half = core // 2, core % 2
        X = np.asarray(q[b], f).reshape(C, S)
        # rotate keys so this core's queries are always columns [0, QH);
        # softmax is invariant to a consistent key permutation
        Xr = np.concatenate([X[:, half * QH:], X[:, :half * QH]], axis=1)
        x8 = q8(Xr)
        beta = (w_beta @ Xr).astype(f)
        # bb[p, kc] = beta[kc*P+p]; bb[p, NKC+oc] = b2[oc*P+p]
        bb = np.concatenate(
            [beta.reshape(NKC, P).T, b2.reshape(CC, P).T], axis=1)
        in_maps.append({
            "x": x8,
            "xt": np.ascontiguousarray(x8.T),
            "m0": m0, "n0": n0, "ones2": ones2,
            "bb": np.ascontiguousarray(bb),
        })
    return in_maps


def gather_out(per_core_outs):
    out = np.zeros((B, C, S), np.float32)
    for core in range(N_CORES):
        b, half = core // 2, core % 2
        out[b, :, half * QH:(half + 1) * QH] = np.asarray(
            per_core_outs[core]).astype(np.float32)
    return out.reshape(B, C, H, W)


def kernel(q, Wq, bq, Wk, bk, Wv, bv, Wo, bo):
    nc = _get_nc()
    in_maps = make_in_maps(q, Wq, bq, Wk, bk, Wv, bv, Wo, bo)
    res = run_bass_kernel_spmd(nc, in_maps, core_ids=list(range(N_CORES)))
    return gather_out([res.results[i]["out"] for i in range(N_CORES)])
